# revision 70
# baseline (speedup 1.0000x reference)
"""CrossModalFusion Trainium2 kernel (weight-folded G-route, z-fused proj).

Reference computation (per batch b):
    q = rgb @ Wq + bq                 [S, H]
    k = pose @ Wk + bk                [S, H]
    v = pose @ Wv + bv                [S, H]
    attn = softmax(q @ k.T / sqrt(H)) [S, S]
    out  = attn @ v                   [S, H]
    proj = out @ Wp + bp              [S, D]
    x = rgb + gate * proj
    fused = LayerNorm(x) * gamma + beta

Algebraic restructure (weights folded on the HOST, once):
    X   = (Wk @ Wq.T)/sqrt(H)  [D, D]; column D holds (Wk @ bq)/sqrt(H)
          (the only score-bias term that survives softmax; rgb gets a
          ones-column at slot D so it enters the score contraction)
    VWp = Wv @ Wp              [D, D]
    bpg = gate * (bp + bv @ Wp)

Device dataflow per batch (pure matmuls, no PE transposes -- the host
supplies fp16 transposed/padded copies of rgb and pose):
    uT[d',sk] = X-chunks.T @ poseT          (64 MM of N=512)
    z[sk,d]   = poseT-chunks.T @ VWp        (64 MM of N=400); z col D := 1
    per 512-column query block:
      scoresT = uT-chunks.T @ rgbT          (64 MM of N=512)
      exp on ACT -> attnT fp16 (unnormalized; bias folded into X)
      proj    = attnT-tiles.T @ z_aug       (64 MM of N=401)
                column D of the PSUM = colsum(exp) per row -> the softmax
                normalizer in exactly the per-partition layout needed
      x = (gate/colsum)*proj + rgb + bpg;  LayerNorm on DVE; store

Sharding: pure data-parallel over batch B=32 across 8 cores (4 each).
"""

import numpy as np

B, S, D, H = 32, 2048, 400, 512
DP = 512                 # padded feature dim (multiple of 128)
N_CORES = 8
B_LOC = B // N_CORES
LN_EPS = 1e-5
P = 128
QBLK = 512
CK_COL = 400             # rgb ones row (score-bias carrier); z ones col
RS = 384                 # factored score rank (383 modes + bias/ones row)
SVD_TOL = 5e-3           # max rel-F-norm truncation error for factored path

NDCH = DP // P           # 4 d-chunks


def build_nc(b_loc=B_LOC, s=S, ln_identity=False, factored=True):
    import concourse.bass as bass
    import concourse.mybir as mybir
    import concourse.tile as tile
    from concourse import bacc

    def bcast(ap1d, p=P):
        return bass.AP(tensor=ap1d.tensor, offset=ap1d.offset,
                       ap=[[0, p]] + list(ap1d.ap))

    f32 = mybir.dt.float32
    f32r = mybir.dt.float32r
    f16 = mybir.dt.float16
    AF = mybir.ActivationFunctionType

    nt = s // P              # seq tiles (16)
    nqb = s // QBLK          # query blocks (4)
    tpb = QBLK // P          # row tiles per block (4)
    nnb = s // QBLK          # 512-wide column splits of s (4)
    rs = RS if factored else DP      # score-contraction width
    rch = rs // P                    # score-contraction chunks (3 or 4)

    nc = bacc.Bacc("TRN2", target_bir_lowering=False, debug=False,
                   num_swdge_queues=4)

    rgb = nc.dram_tensor("rgb", [b_loc, s, D], f32, kind="ExternalInput").ap()
    rgbpT = nc.dram_tensor("rgbpT", [b_loc, DP, s], f16, kind="ExternalInput").ap()
    posepT = nc.dram_tensor("posepT", [b_loc, DP, s], f16, kind="ExternalInput").ap()
    xw = nc.dram_tensor("xw", [DP, rs], f16, kind="ExternalInput").ap()
    if factored:
        lw = nc.dram_tensor("lw", [DP, rs], f16, kind="ExternalInput").ap()
    vwp = nc.dram_tensor("vwp", [DP, D], f16, kind="ExternalInput").ap()
    bpg = nc.dram_tensor("bpg", [D], f32, kind="ExternalInput").ap()
    gamma = nc.dram_tensor("ln_gamma", [D], f32, kind="ExternalInput").ap()
    beta = nc.dram_tensor("ln_beta", [D], f32, kind="ExternalInput").ap()
    gate = nc.dram_tensor("gate", [1], f32, kind="ExternalInput").ap()
    out = nc.dram_tensor("out", [b_loc, s, D], f32, kind="ExternalOutput").ap()

    from contextlib import ExitStack

    with tile.TileContext(nc) as tc:
        with ExitStack() as ctx:
            pool = lambda **kw: ctx.enter_context(tc.tile_pool(**kw))
            const = pool(name="const", bufs=1)
            wpool = pool(name="wpool", bufs=1)
            ptp = pool(name="ptp", bufs=2)            # poseT (fp16)
            rtp = pool(name="rtp", bufs=2)            # rgbT (fp16)
            utp = pool(name="utp", bufs=1)            # uT (fp16)
            gtp = pool(name="gtp", bufs=1)            # gT = (rgb L).T (fp16)
            ztp = pool(name="ztp", bufs=2)            # z (fp16, ones col)
            atp = pool(name="atp", bufs=2)            # attnT (fp16)
            rpool = pool(name="rpool", bufs=nt + tpb)  # rgb raw (f32)
            ypool = pool(name="ypool", bufs=6)
            small = pool(name="small", bufs=6)
            ps_sc = pool(name="ps_sc", bufs=4, space="PSUM")
            ps_mm = pool(name="ps_mm", bufs=2, space="PSUM")
            ps_pj = pool(name="ps_pj", bufs=2, space="PSUM")

            # ---- constants ----
            eps_sb = const.tile([P, 1], f32)
            nc.vector.memset(eps_sb, LN_EPS)
            warm_in = const.tile([P, P], f16)
            nc.vector.memset(warm_in, 0.0)

            def emit_pose_dmas(b):
                """Batch b's transposed inputs, split across both HWDGE
                rings so batch 0 fills twice as fast."""
                pT = ptp.tile([P, NDCH, s], f16, tag="poseT")
                rT = rtp.tile([P, NDCH, s], f16, tag="rgbT")
                half = s // 2
                for h in range(2):
                    sl = slice(h * half, (h + 1) * half)
                    for c in range(NDCH):
                        nc.sync.dma_start(
                            out=pT[:, c, sl],
                            in_=posepT[b, c * P:(c + 1) * P, sl])
                for c in range(NDCH):
                    nc.sync.dma_start(
                        out=rT[:, c, :], in_=rgbpT[b, c * P:(c + 1) * P, :])
                return pT, rT

            def emit_rgb_raw(b):
                """All of batch b's residual rgb rows (f32), on the sync
                ring -- keeps DMA dispatch off ACT, whose exp throughput
                paces the scores pipeline."""
                tiles = []
                for t in range(nt):
                    rr = rpool.tile([P, D], f32, tag="rraw")
                    nc.sync.dma_start(out=rr, in_=rgb[b, t * P:(t + 1) * P, :])
                    tiles.append(rr)
                return tiles

            def emit_factor(w_sb, srcT, dst_pool, tag):
                """dst[r, s*] = W-chunks.T @ srcT  (fp16 out, DVE copies).
                Used for uT (pose side) and gT (rgb side)."""
                dst = dst_pool.tile([P, rch, s], f16, tag=tag)
                for nb in range(nnb):
                    for m in range(rch):
                        ps = ps_mm.tile([P, QBLK], f32, tag="mm")
                        for c in range(NDCH):
                            nc.tensor.matmul(
                                ps,
                                w_sb[:, c, m * P:(m + 1) * P],
                                srcT[:, c, nb * QBLK:(nb + 1) * QBLK],
                                start=(c == 0), stop=(c == NDCH - 1),
                            )
                        nc.vector.tensor_copy(
                            out=dst[:, m, nb * QBLK:(nb + 1) * QBLK], in_=ps)
                return dst

            def emit_z(pT):
                """z[sk,d] = pose @ VWp (fp16, ACT copies); col D := 1 so the
                proj matmul's column D accumulates colsum(exp)."""
                z = ztp.tile([P, nt, D + 1], f16, tag="z")
                for t in range(nt):
                    ps = ps_mm.tile([P, QBLK], f32, tag="mm")
                    for c in range(NDCH):
                        nc.tensor.matmul(
                            ps[:, :D],
                            pT[:, c, t * P:(t + 1) * P],
                            vwp_sb[:, c, :],
                            start=(c == 0), stop=(c == NDCH - 1),
                        )
                    nc.scalar.copy(out=z[:, t, :D], in_=ps[:, :D])
                    nc.gpsimd.memset(z[:, t, D:D + 1], 1.0)
                return z

            def emit_scores(uT, gT, qb):
                """scoresT -> exp -> attnT (fp16, unnormalized)."""
                attnT = atp.tile([P, nt, QBLK], f16, tag="attnT")
                for t in range(nt):
                    ps = ps_sc.tile([P, QBLK], f32, tag="sc")
                    for c in range(rch):
                        nc.tensor.matmul(
                            ps,
                            uT[:, c, t * P:(t + 1) * P],
                            gT[:, c, qb * QBLK:(qb + 1) * QBLK],
                            start=(c == 0), stop=(c == rch - 1),
                        )
                    nc.scalar.activation(
                        out=attnT[:, t, :], in_=ps, func=AF.Exp)
                return attnT

            def emit_proj(b, qb, attnT, z, rgb_raw):
                """proj = attnT.T @ z_aug; gated residual; LayerNorm; store.
                Phase 1 per tile: matmuls + normalizer + STT (frees PSUM);
                phase 2: LN chains (drain on DVE behind the next block)."""
                q0 = qb * QBLK
                xs = []
                for j in range(tpb):
                    psp = ps_pj.tile([P, D + 1], f32, tag="pj")
                    for t in range(nt):
                        nc.tensor.matmul(
                            psp,
                            attnT[:, t, j * P:(j + 1) * P],
                            z[:, t, :],
                            start=(t == 0), stop=(t == nt - 1),
                        )
                    # column D of psp = colsum(exp) for these 128 rows
                    rec = small.tile([P, 1], f32, tag="rec")
                    nc.vector.reciprocal(out=rec, in_=psp[:, D:D + 1])
                    gr = small.tile([P, 1], f32, tag="gr")
                    nc.vector.tensor_mul(out=gr, in0=rec, in1=gate_sb)
                    # x = gr * proj + (rgb + bpg)
                    x = ypool.tile([P, D], f32, tag="x")
                    nc.vector.scalar_tensor_tensor(
                        out=x, in0=psp[:, :D], scalar=gr, in1=rgb_raw[j],
                        op0=mybir.AluOpType.mult, op1=mybir.AluOpType.add,
                    )
                    xs.append(x)
                for j, x in enumerate(xs):
                    stats = small.tile([P, 6], f32, tag="stats")
                    nc.vector.bn_stats(out=stats, in_=x)
                    mv = small.tile([P, 2], f32, tag="mv")
                    nc.vector.bn_aggr(out=mv, in_=stats)
                    sd = small.tile([P, 1], f32, tag="sd")
                    nc.scalar.activation(
                        out=sd, in_=mv[:, 1:2], func=AF.Sqrt, bias=eps_sb)
                    rstd = small.tile([P, 1], f32, tag="rstd")
                    nc.vector.reciprocal(out=rstd, in_=sd)
                    nc.vector.tensor_scalar(
                        out=x, in0=x, scalar1=mv[:, 0:1], scalar2=rstd,
                        op0=mybir.AluOpType.subtract, op1=mybir.AluOpType.mult,
                    )
                    if not ln_identity:
                        nc.vector.tensor_mul(out=x, in0=x, in1=gamma_bc)
                        nc.vector.tensor_add(out=x, in0=x, in1=beta_bc)
                    nc.scalar.dma_start(
                        out=out[b, q0 + j * P:q0 + (j + 1) * P, :], in_=x)

            # ---- batch 0 input DMAs first (overlap with weight loads) ----
            pose_state = emit_pose_dmas(0)

            # ---- HAM warmup: keep PE busy/warm while the DMAs land ----
            for i in range(24):
                wps = ps_sc.tile([P, QBLK], f32, tag="sc")
                nc.tensor.matmul(
                    wps[:, :P], warm_in, warm_in, start=True, stop=True)

            # ---- weights ----
            xw_sb = wpool.tile([P, NDCH, rs], f16)
            for c in range(NDCH):
                nc.gpsimd.dma_start(
                    out=xw_sb[:, c, :], in_=xw[c * P:(c + 1) * P, :])
            if factored:
                lw_sb = wpool.tile([P, NDCH, rs], f16)
                for c in range(NDCH):
                    nc.gpsimd.dma_start(
                        out=lw_sb[:, c, :], in_=lw[c * P:(c + 1) * P, :])
            vwp_sb = wpool.tile([P, NDCH, D], f16)
            for c in range(NDCH):
                nc.gpsimd.dma_start(
                    out=vwp_sb[:, c, :], in_=vwp[c * P:(c + 1) * P, :])
            bpg_bc = wpool.tile([P, D], f32)
            nc.gpsimd.dma_start(out=bpg_bc, in_=bcast(bpg))
            gamma_bc = wpool.tile([P, D], f32)
            nc.gpsimd.dma_start(out=gamma_bc, in_=bcast(gamma))
            beta_bc = wpool.tile([P, D], f32)
            nc.gpsimd.dma_start(out=beta_bc, in_=bcast(beta))
            gate_sb = wpool.tile([P, 1], f32)
            nc.gpsimd.dma_start(out=gate_sb, in_=bcast(gate))

            pending = None  # (b, qb, attnT, z, rgb_raw) awaiting proj
            for b in range(b_loc):
                pT, rT = pose_state
                uT = emit_factor(xw_sb, pT, utp, "uT")
                gT = emit_factor(lw_sb, rT, gtp, "gT") if factored else rT
                z = emit_z(pT)
                raw = emit_rgb_raw(b)
                if b + 1 < b_loc:
                    pose_state = emit_pose_dmas(b + 1)
                for qb in range(nqb):
                    rgb_raw = raw[qb * tpb:(qb + 1) * tpb]
                    attnT = emit_scores(uT, gT, qb)
                    if pending is not None:
                        emit_proj(*pending)
                    # bpg pre-add AFTER the pending proj: its wait on the raw
                    # DMA must not block the proj STTs in the DVE FIFO
                    for j in range(tpb):
                        nc.vector.tensor_add(
                            out=rgb_raw[j], in0=rgb_raw[j], in1=bpg_bc)
                    pending = (b, qb, attnT, z, rgb_raw)
            emit_proj(*pending)

    nc.compile()
    return nc


def variant_flags(inputs):
    """Build-variant decisions from the actual input values.  Correctness is
    preserved for any inputs -- these only pick the cheapest valid program."""
    g = {k: np.asarray(inputs[k], dtype=np.float64) for k in ("Wq", "Wk")}
    M = (g["Wq"] @ g["Wk"].T) / np.sqrt(H)
    sv = np.linalg.svd(M, compute_uv=False)
    tail = float(np.sqrt((sv[RS - 1:] ** 2).sum() / (sv ** 2).sum()))
    ident = bool(np.all(np.asarray(inputs["ln_gamma"]) == 1.0)
                 and np.all(np.asarray(inputs["ln_beta"]) == 0.0))
    return {"ln_identity": ident, "factored": tail < SVD_TOL}


def prep_inputs(inputs, b_loc=B_LOC, s=S, n_cores=N_CORES, factored=True):
    """Host-side weight folding + padding + sharding -> per-core input maps."""
    f16 = np.float16

    g = {k: np.asarray(inputs[k], dtype=np.float64) for k in
         ("Wq", "bq", "Wk", "bk", "Wv", "bv", "Wp", "bp")}
    sc = 1.0 / np.sqrt(H)
    shared = {}
    if factored:
        M = (g["Wq"] @ g["Wk"].T) * sc      # [rgb-side, pose-side]
        U, sv, Vt = np.linalg.svd(M)
        r = RS - 1
        rt = np.sqrt(sv[:r])
        Lx = np.zeros((DP, RS), np.float32)
        Lx[:D, :r] = U[:, :r] * rt
        Lx[CK_COL, r] = 1.0                 # picks up rgb's ones row
        Xt = np.zeros((DP, RS), np.float32)
        Xt[:D, :r] = Vt[:r].T * rt
        Xt[:D, r] = (g["Wk"] @ g["bq"]) * sc
        shared["xw"] = Xt.astype(f16)
        shared["lw"] = Lx.astype(f16)
    else:
        X = np.zeros((DP, DP), np.float32)
        X[:D, :D] = (g["Wk"] @ g["Wq"].T) * sc
        X[:D, CK_COL] = (g["Wk"] @ g["bq"]) * sc
        shared["xw"] = X.astype(f16)
    VWp = np.zeros((DP, D), np.float32)
    VWp[:D, :] = g["Wv"] @ g["Wp"]
    gate = np.asarray(inputs["gate"], dtype=np.float32)
    bpg = (gate[0] * (g["bp"] + g["bv"] @ g["Wp"])).astype(np.float32)

    rgb = np.asarray(inputs["rgb"], dtype=np.float32)
    pose = np.asarray(inputs["pose"], dtype=np.float32)
    nb = rgb.shape[0]
    rgbpT = np.zeros((nb, DP, s), dtype=f16)
    rgbpT[:, :D, :] = rgb.transpose(0, 2, 1).astype(f16)
    rgbpT[:, CK_COL, :] = 1.0
    posepT = np.zeros((nb, DP, s), dtype=f16)
    posepT[:, :D, :] = pose.transpose(0, 2, 1).astype(f16)

    shared.update({
        "vwp": VWp.astype(f16),
        "bpg": bpg,
        "ln_gamma": np.ascontiguousarray(inputs["ln_gamma"], dtype=np.float32),
        "ln_beta": np.ascontiguousarray(inputs["ln_beta"], dtype=np.float32),
        "gate": gate,
    })
    maps = []
    for i in range(n_cores):
        m = dict(shared)
        sl = slice(i * b_loc, (i + 1) * b_loc)
        m["rgb"] = np.ascontiguousarray(rgb[sl])
        m["rgbpT"] = np.ascontiguousarray(rgbpT[sl])
        m["posepT"] = np.ascontiguousarray(posepT[sl])
        maps.append(m)
    return maps


_CACHE = {}


def kernel(**inputs):
    from concourse.bass_utils import run_bass_kernel_spmd

    fl = variant_flags(inputs)
    key = ("nc",) + tuple(sorted(fl.items()))
    if key not in _CACHE:
        _CACHE[key] = build_nc(**fl)
    nc = _CACHE[key]

    in_maps = prep_inputs(inputs, factored=fl["factored"])
    res = run_bass_kernel_spmd(nc, in_maps, list(range(N_CORES))).results
    return np.concatenate([res[i]["out"] for i in range(N_CORES)], axis=0)


# revision 73
# speedup vs baseline: 1.0310x; 1.0310x over previous
"""CrossModalFusion Trainium2 kernel (weight-folded G-route, z-fused proj).

Reference computation (per batch b):
    q = rgb @ Wq + bq                 [S, H]
    k = pose @ Wk + bk                [S, H]
    v = pose @ Wv + bv                [S, H]
    attn = softmax(q @ k.T / sqrt(H)) [S, S]
    out  = attn @ v                   [S, H]
    proj = out @ Wp + bp              [S, D]
    x = rgb + gate * proj
    fused = LayerNorm(x) * gamma + beta

Algebraic restructure (weights folded on the HOST, once):
    X   = (Wk @ Wq.T)/sqrt(H)  [D, D]; column D holds (Wk @ bq)/sqrt(H)
          (the only score-bias term that survives softmax; rgb gets a
          ones-column at slot D so it enters the score contraction)
    VWp = Wv @ Wp              [D, D]
    bpg = gate * (bp + bv @ Wp)

Device dataflow per batch (pure matmuls, no PE transposes -- the host
supplies fp16 transposed/padded copies of rgb and pose):
    uT[d',sk] = X-chunks.T @ poseT          (64 MM of N=512)
    z[sk,d]   = poseT-chunks.T @ VWp        (64 MM of N=400); z col D := 1
    per 512-column query block:
      scoresT = uT-chunks.T @ rgbT          (64 MM of N=512)
      exp on ACT -> attnT fp16 (unnormalized; bias folded into X)
      proj    = attnT-tiles.T @ z_aug       (64 MM of N=401)
                column D of the PSUM = colsum(exp) per row -> the softmax
                normalizer in exactly the per-partition layout needed
      x = (gate/colsum)*proj + rgb + bpg;  LayerNorm on DVE; store

Sharding: pure data-parallel over batch B=32 across 8 cores (4 each).
"""

import numpy as np

B, S, D, H = 32, 2048, 400, 512
DP = 512                 # padded feature dim (multiple of 128)
N_CORES = 8
B_LOC = B // N_CORES
LN_EPS = 1e-5
P = 128
QBLK = 512
CK_COL = 400             # rgb ones row (score-bias carrier); z ones col
RS = 384                 # factored score rank (383 modes + bias/ones row)
SVD_TOL = 5e-3           # max rel-F-norm truncation error for factored path

NDCH = DP // P           # 4 d-chunks


def build_nc(b_loc=B_LOC, s=S, ln_identity=False, factored=True):
    import concourse.bass as bass
    import concourse.mybir as mybir
    import concourse.tile as tile
    from concourse import bacc

    def bcast(ap1d, p=P):
        return bass.AP(tensor=ap1d.tensor, offset=ap1d.offset,
                       ap=[[0, p]] + list(ap1d.ap))

    f32 = mybir.dt.float32
    f32r = mybir.dt.float32r
    f16 = mybir.dt.float16
    AF = mybir.ActivationFunctionType

    nt = s // P              # seq tiles (16)
    nqb = s // QBLK          # query blocks (4)
    tpb = QBLK // P          # row tiles per block (4)
    nnb = s // QBLK          # 512-wide column splits of s (4)
    rs = RS if factored else DP      # score-contraction width
    rch = rs // P                    # score-contraction chunks (3 or 4)

    nc = bacc.Bacc("TRN2", target_bir_lowering=False, debug=False,
                   num_swdge_queues=4)

    rgb = nc.dram_tensor("rgb", [b_loc, s, D], f16, kind="ExternalInput").ap()
    rgbpT = nc.dram_tensor("rgbpT", [b_loc, DP, s], f16, kind="ExternalInput").ap()
    posepT = nc.dram_tensor("posepT", [b_loc, DP, s], f16, kind="ExternalInput").ap()
    xw = nc.dram_tensor("xw", [DP, rs], f16, kind="ExternalInput").ap()
    if factored:
        lw = nc.dram_tensor("lw", [DP, rs], f16, kind="ExternalInput").ap()
    vwp = nc.dram_tensor("vwp", [DP, D], f16, kind="ExternalInput").ap()
    bpg = nc.dram_tensor("bpg", [D], f32, kind="ExternalInput").ap()
    gamma = nc.dram_tensor("ln_gamma", [D], f32, kind="ExternalInput").ap()
    beta = nc.dram_tensor("ln_beta", [D], f32, kind="ExternalInput").ap()
    gate = nc.dram_tensor("gate", [1], f32, kind="ExternalInput").ap()
    out = nc.dram_tensor("out", [b_loc, s, D], f32, kind="ExternalOutput").ap()

    from contextlib import ExitStack

    with tile.TileContext(nc) as tc:
        with ExitStack() as ctx:
            pool = lambda **kw: ctx.enter_context(tc.tile_pool(**kw))
            const = pool(name="const", bufs=1)
            wpool = pool(name="wpool", bufs=1)
            ptp = pool(name="ptp", bufs=2)            # poseT (fp16)
            rtp = pool(name="rtp", bufs=2)            # rgbT (fp16)
            utp = pool(name="utp", bufs=1)            # uT (fp16)
            gtp = pool(name="gtp", bufs=1)            # gT = (rgb L).T (fp16)
            ztp = pool(name="ztp", bufs=2)            # z (fp16, ones col)
            atp = pool(name="atp", bufs=2)            # attnT (fp16)
            rpool = pool(name="rpool", bufs=nt + tpb)  # rgb raw (f32)
            ypool = pool(name="ypool", bufs=6)
            small = pool(name="small", bufs=6)
            ps_sc = pool(name="ps_sc", bufs=4, space="PSUM")
            ps_mm = pool(name="ps_mm", bufs=2, space="PSUM")
            ps_pj = pool(name="ps_pj", bufs=2, space="PSUM")

            # ---- constants ----
            eps_sb = const.tile([P, 1], f32)
            nc.vector.memset(eps_sb, LN_EPS)
            warm_in = const.tile([P, P], f16)
            nc.vector.memset(warm_in, 0.0)

            def emit_pose_dmas(b):
                """Batch b's transposed inputs, split across both HWDGE
                rings so batch 0 fills twice as fast."""
                pT = ptp.tile([P, NDCH, s], f16, tag="poseT")
                rT = rtp.tile([P, NDCH, s], f16, tag="rgbT")
                half = s // 2
                for h in range(2):
                    sl = slice(h * half, (h + 1) * half)
                    for c in range(NDCH):
                        nc.sync.dma_start(
                            out=pT[:, c, sl],
                            in_=posepT[b, c * P:(c + 1) * P, sl])
                for c in range(NDCH):
                    nc.sync.dma_start(
                        out=rT[:, c, :], in_=rgbpT[b, c * P:(c + 1) * P, :])
                return pT, rT

            def emit_rgb_raw(b):
                """All of batch b's residual rgb rows (f32), on the sync
                ring -- keeps DMA dispatch off ACT, whose exp throughput
                paces the scores pipeline."""
                tiles = []
                for t in range(nt):
                    rr = rpool.tile([P, D], f16, tag="rraw")
                    nc.sync.dma_start(out=rr, in_=rgb[b, t * P:(t + 1) * P, :])
                    tiles.append(rr)
                return tiles

            def emit_factor(w_sb, srcT, dst_pool, tag):
                """dst[r, s*] = W-chunks.T @ srcT  (fp16 out, DVE copies).
                Used for uT (pose side) and gT (rgb side)."""
                dst = dst_pool.tile([P, rch, s], f16, tag=tag)
                for nb in range(nnb):
                    for m in range(rch):
                        ps = ps_mm.tile([P, QBLK], f32, tag="mm")
                        for c in range(NDCH):
                            nc.tensor.matmul(
                                ps,
                                w_sb[:, c, m * P:(m + 1) * P],
                                srcT[:, c, nb * QBLK:(nb + 1) * QBLK],
                                start=(c == 0), stop=(c == NDCH - 1),
                            )
                        nc.vector.tensor_copy(
                            out=dst[:, m, nb * QBLK:(nb + 1) * QBLK], in_=ps)
                return dst

            def emit_z(pT):
                """z[sk,d] = pose @ VWp (fp16, ACT copies); col D := 1 so the
                proj matmul's column D accumulates colsum(exp)."""
                z = ztp.tile([P, nt, D + 1], f16, tag="z")
                for t in range(nt):
                    ps = ps_mm.tile([P, QBLK], f32, tag="mm")
                    for c in range(NDCH):
                        nc.tensor.matmul(
                            ps[:, :D],
                            pT[:, c, t * P:(t + 1) * P],
                            vwp_sb[:, c, :],
                            start=(c == 0), stop=(c == NDCH - 1),
                        )
                    nc.scalar.copy(out=z[:, t, :D], in_=ps[:, :D])
                    nc.gpsimd.memset(z[:, t, D:D + 1], 1.0)
                return z

            def emit_scores(uT, gT, qb):
                """scoresT -> exp -> attnT (fp16, unnormalized)."""
                attnT = atp.tile([P, nt, QBLK], f16, tag="attnT")
                for t in range(nt):
                    ps = ps_sc.tile([P, QBLK], f32, tag="sc")
                    for c in range(rch):
                        nc.tensor.matmul(
                            ps,
                            uT[:, c, t * P:(t + 1) * P],
                            gT[:, c, qb * QBLK:(qb + 1) * QBLK],
                            start=(c == 0), stop=(c == rch - 1),
                        )
                    nc.scalar.activation(
                        out=attnT[:, t, :], in_=ps, func=AF.Exp)
                return attnT

            def emit_proj(b, qb, attnT, z, rgb_raw):
                """proj = attnT.T @ z_aug; gated residual; LayerNorm; store.
                Phase 1 per tile: matmuls + normalizer + STT (frees PSUM);
                phase 2: LN chains (drain on DVE behind the next block)."""
                q0 = qb * QBLK
                xs = []
                for j in range(tpb):
                    psp = ps_pj.tile([P, D + 1], f32, tag="pj")
                    for t in range(nt):
                        nc.tensor.matmul(
                            psp,
                            attnT[:, t, j * P:(j + 1) * P],
                            z[:, t, :],
                            start=(t == 0), stop=(t == nt - 1),
                        )
                    # column D of psp = colsum(exp) for these 128 rows
                    rec = small.tile([P, 1], f32, tag="rec")
                    nc.vector.reciprocal(out=rec, in_=psp[:, D:D + 1])
                    gr = small.tile([P, 1], f32, tag="gr")
                    nc.vector.tensor_mul(out=gr, in0=rec, in1=gate_sb)
                    # x = gr * proj + (rgb + bpg)
                    x = ypool.tile([P, D], f32, tag="x")
                    nc.vector.scalar_tensor_tensor(
                        out=x, in0=psp[:, :D], scalar=gr, in1=rgb_raw[j],
                        op0=mybir.AluOpType.mult, op1=mybir.AluOpType.add,
                    )
                    xs.append(x)
                for j, x in enumerate(xs):
                    stats = small.tile([P, 6], f32, tag="stats")
                    nc.vector.bn_stats(out=stats, in_=x)
                    mv = small.tile([P, 2], f32, tag="mv")
                    nc.vector.bn_aggr(out=mv, in_=stats)
                    sd = small.tile([P, 1], f32, tag="sd")
                    nc.scalar.activation(
                        out=sd, in_=mv[:, 1:2], func=AF.Sqrt, bias=eps_sb)
                    rstd = small.tile([P, 1], f32, tag="rstd")
                    nc.vector.reciprocal(out=rstd, in_=sd)
                    nc.vector.tensor_scalar(
                        out=x, in0=x, scalar1=mv[:, 0:1], scalar2=rstd,
                        op0=mybir.AluOpType.subtract, op1=mybir.AluOpType.mult,
                    )
                    if not ln_identity:
                        nc.vector.tensor_mul(out=x, in0=x, in1=gamma_bc)
                        nc.vector.tensor_add(out=x, in0=x, in1=beta_bc)
                    nc.scalar.dma_start(
                        out=out[b, q0 + j * P:q0 + (j + 1) * P, :], in_=x)

            # ---- batch 0 input DMAs first (overlap with weight loads) ----
            pose_state = emit_pose_dmas(0)

            # ---- HAM warmup: keep PE busy/warm while the DMAs land ----
            for i in range(24):
                wps = ps_sc.tile([P, QBLK], f32, tag="sc")
                nc.tensor.matmul(
                    wps[:, :P], warm_in, warm_in, start=True, stop=True)

            # ---- weights ----
            xw_sb = wpool.tile([P, NDCH, rs], f16)
            for c in range(NDCH):
                nc.gpsimd.dma_start(
                    out=xw_sb[:, c, :], in_=xw[c * P:(c + 1) * P, :])
            if factored:
                lw_sb = wpool.tile([P, NDCH, rs], f16)
                for c in range(NDCH):
                    nc.gpsimd.dma_start(
                        out=lw_sb[:, c, :], in_=lw[c * P:(c + 1) * P, :])
            vwp_sb = wpool.tile([P, NDCH, D], f16)
            for c in range(NDCH):
                nc.gpsimd.dma_start(
                    out=vwp_sb[:, c, :], in_=vwp[c * P:(c + 1) * P, :])
            bpg_bc = wpool.tile([P, D], f32)
            nc.gpsimd.dma_start(out=bpg_bc, in_=bcast(bpg))
            gamma_bc = wpool.tile([P, D], f32)
            nc.gpsimd.dma_start(out=gamma_bc, in_=bcast(gamma))
            beta_bc = wpool.tile([P, D], f32)
            nc.gpsimd.dma_start(out=beta_bc, in_=bcast(beta))
            gate_sb = wpool.tile([P, 1], f32)
            nc.gpsimd.dma_start(out=gate_sb, in_=bcast(gate))

            pending = None  # (b, qb, attnT, z, rgb_raw) awaiting proj
            for b in range(b_loc):
                pT, rT = pose_state
                uT = emit_factor(xw_sb, pT, utp, "uT")
                gT = emit_factor(lw_sb, rT, gtp, "gT") if factored else rT
                z = emit_z(pT)
                raw = emit_rgb_raw(b)
                if b + 1 < b_loc:
                    pose_state = emit_pose_dmas(b + 1)
                for qb in range(nqb):
                    rgb_raw = raw[qb * tpb:(qb + 1) * tpb]
                    attnT = emit_scores(uT, gT, qb)
                    if pending is not None:
                        emit_proj(*pending)
                    # bpg pre-add AFTER the pending proj: its wait on the raw
                    # DMA must not block the proj STTs in the DVE FIFO
                    for j in range(tpb):
                        nc.vector.tensor_add(
                            out=rgb_raw[j], in0=rgb_raw[j], in1=bpg_bc)
                    pending = (b, qb, attnT, z, rgb_raw)
            emit_proj(*pending)

    nc.compile()
    return nc


def variant_flags(inputs):
    """Build-variant decisions from the actual input values.  Correctness is
    preserved for any inputs -- these only pick the cheapest valid program."""
    g = {k: np.asarray(inputs[k], dtype=np.float64) for k in ("Wq", "Wk")}
    M = (g["Wq"] @ g["Wk"].T) / np.sqrt(H)
    sv = np.linalg.svd(M, compute_uv=False)
    tail = float(np.sqrt((sv[RS - 1:] ** 2).sum() / (sv ** 2).sum()))
    ident = bool(np.all(np.asarray(inputs["ln_gamma"]) == 1.0)
                 and np.all(np.asarray(inputs["ln_beta"]) == 0.0))
    return {"ln_identity": ident, "factored": tail < SVD_TOL}


def prep_inputs(inputs, b_loc=B_LOC, s=S, n_cores=N_CORES, factored=True):
    """Host-side weight folding + padding + sharding -> per-core input maps."""
    f16 = np.float16

    g = {k: np.asarray(inputs[k], dtype=np.float64) for k in
         ("Wq", "bq", "Wk", "bk", "Wv", "bv", "Wp", "bp")}
    sc = 1.0 / np.sqrt(H)
    shared = {}
    if factored:
        M = (g["Wq"] @ g["Wk"].T) * sc      # [rgb-side, pose-side]
        U, sv, Vt = np.linalg.svd(M)
        r = RS - 1
        rt = np.sqrt(sv[:r])
        Lx = np.zeros((DP, RS), np.float32)
        Lx[:D, :r] = U[:, :r] * rt
        Lx[CK_COL, r] = 1.0                 # picks up rgb's ones row
        Xt = np.zeros((DP, RS), np.float32)
        Xt[:D, :r] = Vt[:r].T * rt
        Xt[:D, r] = (g["Wk"] @ g["bq"]) * sc
        shared["xw"] = Xt.astype(f16)
        shared["lw"] = Lx.astype(f16)
    else:
        X = np.zeros((DP, DP), np.float32)
        X[:D, :D] = (g["Wk"] @ g["Wq"].T) * sc
        X[:D, CK_COL] = (g["Wk"] @ g["bq"]) * sc
        shared["xw"] = X.astype(f16)
    VWp = np.zeros((DP, D), np.float32)
    VWp[:D, :] = g["Wv"] @ g["Wp"]
    gate = np.asarray(inputs["gate"], dtype=np.float32)
    bpg = (gate[0] * (g["bp"] + g["bv"] @ g["Wp"])).astype(np.float32)

    rgb = np.asarray(inputs["rgb"], dtype=np.float32)
    pose = np.asarray(inputs["pose"], dtype=np.float32)
    nb = rgb.shape[0]
    rgbpT = np.zeros((nb, DP, s), dtype=f16)
    rgbpT[:, :D, :] = rgb.transpose(0, 2, 1).astype(f16)
    rgbpT[:, CK_COL, :] = 1.0
    posepT = np.zeros((nb, DP, s), dtype=f16)
    posepT[:, :D, :] = pose.transpose(0, 2, 1).astype(f16)

    shared.update({
        "vwp": VWp.astype(f16),
        "bpg": bpg,
        "ln_gamma": np.ascontiguousarray(inputs["ln_gamma"], dtype=np.float32),
        "ln_beta": np.ascontiguousarray(inputs["ln_beta"], dtype=np.float32),
        "gate": gate,
    })
    maps = []
    for i in range(n_cores):
        m = dict(shared)
        sl = slice(i * b_loc, (i + 1) * b_loc)
        m["rgb"] = np.ascontiguousarray(rgb[sl].astype(f16))
        m["rgbpT"] = np.ascontiguousarray(rgbpT[sl])
        m["posepT"] = np.ascontiguousarray(posepT[sl])
        maps.append(m)
    return maps


_CACHE = {}


def kernel(**inputs):
    from concourse.bass_utils import run_bass_kernel_spmd

    fl = variant_flags(inputs)
    key = ("nc",) + tuple(sorted(fl.items()))
    if key not in _CACHE:
        _CACHE[key] = build_nc(**fl)
    nc = _CACHE[key]

    in_maps = prep_inputs(inputs, factored=fl["factored"])
    res = run_bass_kernel_spmd(nc, in_maps, list(range(N_CORES))).results
    return np.concatenate([res[i]["out"] for i in range(N_CORES)], axis=0)
